# revision 4
# baseline (speedup 1.0000x reference)
"""Trainium2 Bass kernel for AdaptiveLinearWithChannel (moe_routing).

Reference computation:
    w = weight[indices, t]          # (N_sel, D_in, D_out)
    b = bias[indices, t]            # (N_sel, 1, D_out)
    out = x @ w + b                 # (N_sel, PTS, D_out)

Sharding: the selected-channel dim N_sel=256 is split across 8 NeuronCores
(32 channels each, expert-parallel).  The per-channel weight/bias gather is
part of host-side sharding prep; each core then runs 32 independent
(2048x256)@(256x256) GEMMs + bias.

Device layout: the TensorEngine contracts along the partition axis, so x is
staged per-channel as x.T (D_in on partitions).  Each matmul computes an
out.T tile [D_out=128, pts=512] in PSUM (w-slice stationary, x.T moving),
bias is added by VectorE on the way out of PSUM, and the kernel writes out.T
per channel; the host transposes back when unsharding.
"""

import sys

import numpy as np

try:
    import concourse.bacc as bacc
except ImportError:  # fresh dir without the nix sitecustomize on sys.path
    sys.path.insert(0, "/opt/trn_rl_repo")
    import concourse.bacc as bacc

import concourse.mybir as mybir
import concourse.tile as tile
from concourse.bass_utils import run_bass_kernel_spmd

N_SEL = 256
PTS = 2048
D_IN = 256
D_OUT = 256
N_CORES = 8
NCH = N_SEL // N_CORES  # channels per core
P = 128  # partitions

# Compute mode: "f32" (exact), "f32r" (full-rate fp32, reduced mult precision)
COMPUTE = "f32r"
TRACE = False  # test.py flips this to get exec_time_ns

LAST_EXEC_TIME_NS = None

_CACHE = {}


def _build(compute: str):
    f32 = mybir.dt.float32
    mm_dt = mybir.dt.float32r if compute == "f32r" else f32

    nc = bacc.Bacc(None, target_bir_lowering=False)
    xT_ext = nc.declare_dram_parameter("xT", [NCH, D_IN, PTS], mm_dt, isOutput=False)
    w_ext = nc.declare_dram_parameter("w", [NCH, D_IN, D_OUT], mm_dt, isOutput=False)
    bT_ext = nc.declare_dram_parameter("bT", [D_OUT, NCH], f32, isOutput=False)
    out_ext = nc.declare_dram_parameter("outT", [NCH, D_OUT, PTS], f32, isOutput=True)

    KH = D_IN // P  # 2 contraction halves
    MH = D_OUT // P  # 2 output-partition halves
    NPC = PTS // 512  # 4 moving chunks of 512

    with tile.TileContext(nc) as tc:
        with (
            tc.tile_pool(name="xp", bufs=2) as xpool,
            tc.tile_pool(name="wp", bufs=2) as wpool,
            tc.tile_pool(name="bp", bufs=1) as bpool,
            tc.tile_pool(name="op", bufs=8) as opool,
            tc.tile_pool(name="pp", bufs=8, space="PSUM") as pspool,
        ):
            b_sb = bpool.tile([P, MH, NCH], f32, tag="b", name="b_sb")
            for mh in range(MH):
                nc.sync.dma_start(b_sb[:, mh, :], bT_ext[mh * P : (mh + 1) * P, :])

            for ch in range(NCH):
                x_sb = xpool.tile([P, KH, PTS], mm_dt, tag="x", name=f"x{ch}")
                w_sb = wpool.tile([P, KH, D_OUT], mm_dt, tag="w", name=f"w{ch}")
                for kh in range(KH):
                    nc.sync.dma_start(
                        x_sb[:, kh, :], xT_ext[ch, kh * P : (kh + 1) * P, :]
                    )
                    nc.sync.dma_start(
                        w_sb[:, kh, :], w_ext[ch, kh * P : (kh + 1) * P, :]
                    )
                for mh in range(MH):
                    ps = [
                        pspool.tile(
                            [P, 512], f32, tag="ps", name=f"ps{ch}_{mh}_{pc}"
                        )
                        for pc in range(NPC)
                    ]
                    for kh in range(KH):
                        lhsT = w_sb[:, kh, mh * P : (mh + 1) * P]
                        for pc in range(NPC):
                            nc.tensor.matmul(
                                ps[pc][:, :],
                                lhsT,
                                x_sb[:, kh, pc * 512 : (pc + 1) * 512],
                                start=(kh == 0),
                                stop=(kh == KH - 1),
                            )
                    for pc in range(NPC):
                        o_sb = opool.tile([P, 512], f32, tag="o", name=f"o{ch}_{mh}_{pc}")
                        nc.vector.tensor_scalar_add(
                            o_sb[:, :], ps[pc][:, :], b_sb[:, mh, ch : ch + 1]
                        )
                        nc.sync.dma_start(
                            out_ext[
                                ch, mh * P : (mh + 1) * P, pc * 512 : (pc + 1) * 512
                            ],
                            o_sb[:, :],
                        )

    nc.compile()
    return nc


def _install_ntff_hook():
    """The agent image's antenv lacks axon_hooks; register the NTFF
    profiling hook ourselves so trace=True yields exec_time_ns."""
    try:
        from antenv.axon_hooks import get_axon_ntff_profile_hook  # noqa: F401

        return
    except ImportError:
        pass
    import types

    from trn_agent_boot.trn_boot import _ntff_profile_via_ctypes

    hook = _ntff_profile_via_ctypes("/opt/axon/libaxon_pjrt.so")
    mod = types.ModuleType("antenv.axon_hooks")
    mod.get_axon_ntff_profile_hook = lambda: hook
    mod.set_axon_ntff_profile_hook = lambda h: None
    sys.modules["antenv.axon_hooks"] = mod


def _round_tf32(a):
    """Round-to-nearest-even to the 10-bit mantissa the PE's FP32r
    (tfloat32) mode multiplies at."""
    u = a.view(np.uint32)
    r = (u + np.uint32(0xFFF) + ((u >> np.uint32(13)) & np.uint32(1))) & np.uint32(
        0xFFFFE000
    )
    return r.view(np.float32)


def kernel(x, weight, bias, indices, t):
    global LAST_EXEC_TIME_NS

    x = np.asarray(x, dtype=np.float32)
    weight = np.asarray(weight, dtype=np.float32)
    bias = np.asarray(bias, dtype=np.float32)
    idx = np.asarray(indices).astype(np.int64)
    t = int(np.asarray(t))

    # Host-side sharding prep: per-channel gather + transpose.
    w_sel = weight[idx, t]  # (N_sel, D_in, D_out)
    b_sel = bias[idx, t, 0]  # (N_sel, D_out)
    xT = np.ascontiguousarray(x.transpose(0, 2, 1))  # (N_sel, D_in, PTS)
    if COMPUTE == "f32r":
        xT = _round_tf32(xT)
        w_sel = _round_tf32(np.ascontiguousarray(w_sel))

    in_maps = []
    for c in range(N_CORES):
        sl = slice(c * NCH, (c + 1) * NCH)
        in_maps.append(
            {
                "xT": xT[sl],
                "w": np.ascontiguousarray(w_sel[sl]),
                "bT": np.ascontiguousarray(b_sel[sl].T),
            }
        )

    if COMPUTE not in _CACHE:
        _CACHE[COMPUTE] = _build(COMPUTE)
    nc = _CACHE[COMPUTE]

    if TRACE:
        _install_ntff_hook()
    res = run_bass_kernel_spmd(
        nc, in_maps, core_ids=list(range(N_CORES)), trace=TRACE
    )
    LAST_EXEC_TIME_NS = res.exec_time_ns

    outT = np.concatenate(
        [res.results[i]["outT"] for i in range(N_CORES)], axis=0
    )  # (N_sel, D_out, PTS)
    return np.ascontiguousarray(outT.transpose(0, 2, 1))


# revision 6
# speedup vs baseline: 1.5997x; 1.5997x over previous
"""Trainium2 Bass kernel for AdaptiveLinearWithChannel (moe_routing).

Reference computation:
    w = weight[indices, t]          # (N_sel, D_in, D_out)
    b = bias[indices, t]            # (N_sel, 1, D_out)
    out = x @ w + b                 # (N_sel, PTS, D_out)

Sharding: the selected-channel dim N_sel=256 is split across 8 NeuronCores
(32 channels each, expert-parallel).  The per-channel weight/bias gather is
part of host-side sharding prep; each core then runs 32 independent
(2048x256)@(256x256) GEMMs + bias.

Device layout: the TensorEngine contracts along the partition axis, so x is
staged per-channel as x.T (D_in on partitions).  Each matmul computes an
out.T tile [D_out=128, pts=512] in PSUM (w-slice stationary, x.T moving),
bias is added by VectorE on the way out of PSUM, and the kernel writes out.T
per channel; the host transposes back when unsharding.
"""

import sys

import numpy as np

try:
    import concourse.bacc as bacc
except ImportError:  # fresh dir without the nix sitecustomize on sys.path
    sys.path.insert(0, "/opt/trn_rl_repo")
    import concourse.bacc as bacc

import concourse.mybir as mybir
import concourse.tile as tile
from concourse.bass_utils import run_bass_kernel_spmd

N_SEL = 256
PTS = 2048
D_IN = 256
D_OUT = 256
N_CORES = 8
NCH = N_SEL // N_CORES  # channels per core
P = 128  # partitions

# Compute mode: "f32" (exact), "f32r" (full-rate fp32, reduced mult precision)
COMPUTE = "f32r"
TRACE = False  # test.py flips this to get exec_time_ns

LAST_EXEC_TIME_NS = None

_CACHE = {}


def _mm_dtype(compute: str):
    return {
        "f32": mybir.dt.float32,
        "f32r": mybir.dt.float32r,
        "bf16": mybir.dt.bfloat16,
    }[compute]


def _build(compute: str):
    f32 = mybir.dt.float32
    mm_dt = _mm_dtype(compute)

    nc = bacc.Bacc(None, target_bir_lowering=False)
    # xT: per-channel x transposed, [ch, k, pts]
    xT_ext = nc.declare_dram_parameter("xT", [NCH, D_IN, PTS], mm_dt, isOutput=False)
    # w laid out [kh, p, ch, dout] so the whole-table preload gets 32KB
    # contiguous runs per partition
    w_ext = nc.declare_dram_parameter(
        "w", [D_IN // P, P, NCH, D_OUT], mm_dt, isOutput=False
    )
    bT_ext = nc.declare_dram_parameter("bT", [D_OUT, NCH], f32, isOutput=False)
    out_ext = nc.declare_dram_parameter("outT", [NCH, D_OUT, PTS], f32, isOutput=True)

    KH = D_IN // P  # 2 contraction halves
    MH = D_OUT // P  # 2 output-partition halves
    NPC = PTS // 512  # 4 moving chunks of 512

    with tile.TileContext(nc) as tc:
        with (
            tc.tile_pool(name="xp", bufs=3) as xpool,
            tc.tile_pool(name="wp", bufs=1) as wpool,
            tc.tile_pool(name="bp", bufs=1) as bpool,
            tc.tile_pool(name="op", bufs=4) as opool,
            tc.tile_pool(name="pp", bufs=8, space="PSUM") as pspool,
        ):
            # Preloads ride the SWDGE (gpsimd) queue so they never
            # head-of-line-block the streaming x loads on the sync HWDGE ring.
            b_sb = bpool.tile([P, MH, NCH], f32, tag="b", name="b_sb")
            for mh in range(MH):
                nc.gpsimd.dma_start(b_sb[:, mh, :], bT_ext[mh * P : (mh + 1) * P, :])
            w_all = wpool.tile([P, KH, NCH, D_OUT], mm_dt, tag="w", name="w_all")
            for kh in range(KH):
                nc.gpsimd.dma_start(w_all[:, kh, :, :], w_ext[kh, :, :, :])

            for ch in range(NCH):
                # x loads: sync HWDGE ring; 8KB contiguous per partition row
                x_sb = xpool.tile([P, KH, PTS], mm_dt, tag="x", name=f"x{ch}")
                for kh in range(KH):
                    nc.sync.dma_start(
                        x_sb[:, kh, :], xT_ext[ch, kh * P : (kh + 1) * P, :]
                    )
                for mh in range(MH):
                    o_sb = opool.tile([P, PTS], f32, tag="o", name=f"o{ch}_{mh}")
                    ps = [
                        pspool.tile([P, 512], f32, tag="ps", name=f"ps{ch}_{mh}_{pc}")
                        for pc in range(NPC)
                    ]
                    for kh in range(KH):
                        lhsT = w_all[:, kh, ch, mh * P : (mh + 1) * P]
                        for pc in range(NPC):
                            nc.tensor.matmul(
                                ps[pc][:, :],
                                lhsT,
                                x_sb[:, kh, pc * 512 : (pc + 1) * 512],
                                start=(kh == 0),
                                stop=(kh == KH - 1),
                            )
                    for pc in range(NPC):
                        nc.vector.tensor_scalar_add(
                            o_sb[:, pc * 512 : (pc + 1) * 512],
                            ps[pc][:, :],
                            b_sb[:, mh, ch : ch + 1],
                        )
                    # out stores: scalar HWDGE ring (decoupled from loads);
                    # 1MB contiguous DRAM region, 8KB runs per partition
                    nc.scalar.dma_start(
                        out_ext[ch, mh * P : (mh + 1) * P, :], o_sb[:, :]
                    )

    nc.compile()
    return nc


def _install_ntff_hook():
    """The agent image's antenv lacks axon_hooks; register the NTFF
    profiling hook ourselves so trace=True yields exec_time_ns."""
    try:
        from antenv.axon_hooks import get_axon_ntff_profile_hook  # noqa: F401

        return
    except ImportError:
        pass
    import types

    from trn_agent_boot.trn_boot import _ntff_profile_via_ctypes

    hook = _ntff_profile_via_ctypes("/opt/axon/libaxon_pjrt.so")
    mod = types.ModuleType("antenv.axon_hooks")
    mod.get_axon_ntff_profile_hook = lambda: hook
    mod.set_axon_ntff_profile_hook = lambda h: None
    sys.modules["antenv.axon_hooks"] = mod


def _round_tf32(a):
    """Round-to-nearest-even to the 10-bit mantissa the PE's FP32r
    (tfloat32) mode multiplies at."""
    u = a.view(np.uint32)
    r = (u + np.uint32(0xFFF) + ((u >> np.uint32(13)) & np.uint32(1))) & np.uint32(
        0xFFFFE000
    )
    return r.view(np.float32)


def kernel(x, weight, bias, indices, t):
    global LAST_EXEC_TIME_NS

    x = np.asarray(x, dtype=np.float32)
    weight = np.asarray(weight, dtype=np.float32)
    bias = np.asarray(bias, dtype=np.float32)
    idx = np.asarray(indices).astype(np.int64)
    t = int(np.asarray(t))

    # Host-side sharding prep: per-channel gather + transpose.
    w_sel = weight[idx, t]  # (N_sel, D_in, D_out)
    b_sel = bias[idx, t, 0]  # (N_sel, D_out)
    xT = np.ascontiguousarray(x.transpose(0, 2, 1))  # (N_sel, D_in, PTS)
    if COMPUTE == "f32r":
        xT = _round_tf32(xT)
        w_sel = _round_tf32(np.ascontiguousarray(w_sel))
    elif COMPUTE == "bf16":
        import ml_dtypes

        xT = xT.astype(ml_dtypes.bfloat16)
        w_sel = w_sel.astype(ml_dtypes.bfloat16)

    in_maps = []
    for c in range(N_CORES):
        sl = slice(c * NCH, (c + 1) * NCH)
        # w device layout: [kh, p, ch, dout]
        w_dev = np.ascontiguousarray(w_sel[sl].transpose(1, 0, 2)).reshape(
            D_IN // P, P, NCH, D_OUT
        )
        in_maps.append(
            {
                "xT": xT[sl],
                "w": w_dev,
                "bT": np.ascontiguousarray(b_sel[sl].T),
            }
        )

    if COMPUTE not in _CACHE:
        _CACHE[COMPUTE] = _build(COMPUTE)
    nc = _CACHE[COMPUTE]

    if TRACE:
        _install_ntff_hook()
    res = run_bass_kernel_spmd(
        nc, in_maps, core_ids=list(range(N_CORES)), trace=TRACE
    )
    LAST_EXEC_TIME_NS = res.exec_time_ns

    outT = np.concatenate(
        [res.results[i]["outT"] for i in range(N_CORES)], axis=0
    )  # (N_sel, D_out, PTS)
    return np.ascontiguousarray(outT.transpose(0, 2, 1))


# revision 10
# speedup vs baseline: 1.7539x; 1.0964x over previous
"""Trainium2 Bass kernel for AdaptiveLinearWithChannel (moe_routing).

Reference computation:
    w = weight[indices, t]          # (N_sel, D_in, D_out)
    b = bias[indices, t]            # (N_sel, 1, D_out)
    out = x @ w + b                 # (N_sel, PTS, D_out)

Sharding: the selected-channel dim N_sel=256 is split across 8 NeuronCores
(32 channels each, expert-parallel).  The per-channel weight/bias gather is
part of host-side sharding prep; each core then runs 32 independent
(2048x256)@(256x256) GEMMs + bias.

Device layout: the TensorEngine contracts along the partition axis, so x is
staged per-channel as x.T (D_in on partitions).  Each matmul computes an
out.T tile [D_out=128, pts=512] in PSUM (w-slice stationary, x.T moving),
bias is added by VectorE on the way out of PSUM, and the kernel writes out.T
per channel; the host transposes back when unsharding.
"""

import sys

import numpy as np

try:
    import concourse.bacc as bacc
except ImportError:  # fresh dir without the nix sitecustomize on sys.path
    sys.path.insert(0, "/opt/trn_rl_repo")
    import concourse.bacc as bacc

import concourse.mybir as mybir
import concourse.tile as tile
from concourse.bass_utils import run_bass_kernel_spmd

N_SEL = 256
PTS = 2048
D_IN = 256
D_OUT = 256
N_CORES = 8
NCH = N_SEL // N_CORES  # channels per core
P = 128  # partitions

# Compute mode: "f32" (exact), "f32r" (full-rate fp32, reduced mult precision)
COMPUTE = "f32r"
TRACE = False  # test.py flips this to get exec_time_ns

LAST_EXEC_TIME_NS = None

_CACHE = {}


def _mm_dtype(compute: str):
    return {
        "f32": mybir.dt.float32,
        "f32r": mybir.dt.float32r,
        "bf16": mybir.dt.bfloat16,
    }[compute]


def _build(compute: str):
    f32 = mybir.dt.float32
    mm_dt = _mm_dtype(compute)

    KH = D_IN // P  # 2 contraction halves
    MH = D_OUT // P  # 2 output-partition halves
    NPC = PTS // 512  # 4 moving chunks of 512

    nc = bacc.Bacc(None, target_bir_lowering=False)
    # x transposed, [kh, p, ch, pts]: channel-pair loads then read 2*PTS
    # contiguous bytes per partition
    xT_ext = nc.declare_dram_parameter("xT", [KH, P, NCH, PTS], mm_dt, isOutput=False)
    # w laid out [kh, p, ch, dout] so the whole-table preload gets 32KB
    # contiguous runs per partition
    w_ext = nc.declare_dram_parameter("w", [KH, P, NCH, D_OUT], mm_dt, isOutput=False)
    bT_ext = nc.declare_dram_parameter("bT", [D_OUT, NCH], f32, isOutput=False)
    out_ext = nc.declare_dram_parameter(
        "outT", [NCH, D_OUT, NPC, 512], f32, isOutput=True
    )

    CPAIR = 2  # channels loaded per x DMA

    with tile.TileContext(nc) as tc:
        with (
            tc.tile_pool(name="xp", bufs=3 if compute == "bf16" else 2) as xpool,
            tc.tile_pool(name="wp", bufs=1) as wpool,
            tc.tile_pool(name="bp", bufs=1) as bpool,
            tc.tile_pool(name="op", bufs=4) as opool,
            tc.tile_pool(name="pp", bufs=2, space="PSUM") as pspool,
        ):
            # Preloads ride the SWDGE (gpsimd) queue so they never
            # head-of-line-block the streaming x loads on the sync HWDGE ring.
            b_sb = bpool.tile([P, MH, NCH], f32, tag="b", name="b_sb")
            for mh in range(MH):
                nc.gpsimd.dma_start(b_sb[:, mh, :], bT_ext[mh * P : (mh + 1) * P, :])
            w_all = wpool.tile([P, KH, NCH, D_OUT], mm_dt, tag="w", name="w_all")
            for kh in range(KH):
                nc.gpsimd.dma_start(w_all[:, kh, :, :], w_ext[kh, :, :, :])

            for pr in range(NCH // CPAIR):
                # x loads: sync HWDGE ring; CPAIR*PTS contiguous per partition
                x_sb = xpool.tile([P, KH, CPAIR, PTS], mm_dt, tag="x", name=f"x{pr}")
                for kh in range(KH):
                    nc.sync.dma_start(
                        x_sb[:, kh, :, :],
                        xT_ext[kh, :, pr * CPAIR : (pr + 1) * CPAIR, :],
                    )
                for ci in range(CPAIR):
                    ch = pr * CPAIR + ci
                    for mh in range(MH):
                        # one 4-bank PSUM tile per (ch, mh); bufs=2 ping-pongs
                        # across the 8 banks
                        ps4 = pspool.tile(
                            [P, NPC, 512], f32, tag="ps", name=f"ps{ch}_{mh}"
                        )
                        for kh in range(KH):
                            lhsT = w_all[:, kh, ch, mh * P : (mh + 1) * P]
                            for pc in range(NPC):
                                nc.tensor.matmul(
                                    ps4[:, pc, :],
                                    lhsT,
                                    x_sb[:, kh, ci, pc * 512 : (pc + 1) * 512],
                                    start=(kh == 0),
                                    stop=(kh == KH - 1),
                                )
                        # bias add + PSUM evacuation, one 2048-wide op;
                        # alternate VectorE / ScalarE so neither binds
                        o_sb = opool.tile(
                            [P, NPC, 512], f32, tag="o", name=f"o{ch}_{mh}"
                        )
                        bcol = b_sb[:, mh, ch : ch + 1]
                        if mh == 0:
                            nc.vector.tensor_scalar_add(o_sb[:, :, :], ps4[:, :, :], bcol)
                        else:
                            nc.scalar.add(o_sb[:, :, :], ps4[:, :, :], bcol)
                        # out stores: scalar HWDGE ring (decoupled from loads);
                        # 1MB contiguous DRAM region, 8KB runs per partition
                        nc.scalar.dma_start(
                            out_ext[ch, mh * P : (mh + 1) * P, :, :], o_sb[:, :, :]
                        )

    nc.compile()
    return nc


def _install_ntff_hook():
    """The agent image's antenv lacks axon_hooks; register the NTFF
    profiling hook ourselves so trace=True yields exec_time_ns."""
    try:
        from antenv.axon_hooks import get_axon_ntff_profile_hook  # noqa: F401

        return
    except ImportError:
        pass
    import types

    from trn_agent_boot.trn_boot import _ntff_profile_via_ctypes

    hook = _ntff_profile_via_ctypes("/opt/axon/libaxon_pjrt.so")
    mod = types.ModuleType("antenv.axon_hooks")
    mod.get_axon_ntff_profile_hook = lambda: hook
    mod.set_axon_ntff_profile_hook = lambda h: None
    sys.modules["antenv.axon_hooks"] = mod


def _round_tf32(a):
    """Round-to-nearest-even to the 10-bit mantissa the PE's FP32r
    (tfloat32) mode multiplies at."""
    u = a.view(np.uint32)
    r = (u + np.uint32(0xFFF) + ((u >> np.uint32(13)) & np.uint32(1))) & np.uint32(
        0xFFFFE000
    )
    return r.view(np.float32)


def kernel(x, weight, bias, indices, t):
    global LAST_EXEC_TIME_NS

    x = np.asarray(x, dtype=np.float32)
    weight = np.asarray(weight, dtype=np.float32)
    bias = np.asarray(bias, dtype=np.float32)
    idx = np.asarray(indices).astype(np.int64)
    t = int(np.asarray(t))

    # Host-side sharding prep: per-channel gather + transpose + dtype prep.
    w_sel = np.ascontiguousarray(weight[idx, t])  # (N_sel, D_in, D_out)
    b_sel = bias[idx, t, 0]  # (N_sel, D_out)
    if COMPUTE == "f32r":
        x = _round_tf32(x)
        w_sel = _round_tf32(w_sel)
    elif COMPUTE == "bf16":
        import ml_dtypes

        x = x.astype(ml_dtypes.bfloat16)
        w_sel = w_sel.astype(ml_dtypes.bfloat16)

    in_maps = []
    for c in range(N_CORES):
        sl = slice(c * NCH, (c + 1) * NCH)
        # x device layout: [kh, p, ch, pts]
        x_dev = np.ascontiguousarray(x[sl].transpose(2, 0, 1)).reshape(
            D_IN // P, P, NCH, PTS
        )
        # w device layout: [kh, p, ch, dout]
        w_dev = np.ascontiguousarray(w_sel[sl].transpose(1, 0, 2)).reshape(
            D_IN // P, P, NCH, D_OUT
        )
        in_maps.append(
            {
                "xT": x_dev,
                "w": w_dev,
                "bT": np.ascontiguousarray(b_sel[sl].T),
            }
        )

    if COMPUTE not in _CACHE:
        _CACHE[COMPUTE] = _build(COMPUTE)
    nc = _CACHE[COMPUTE]

    if TRACE:
        _install_ntff_hook()
    res = run_bass_kernel_spmd(
        nc, in_maps, core_ids=list(range(N_CORES)), trace=TRACE
    )
    LAST_EXEC_TIME_NS = res.exec_time_ns

    outT = np.concatenate(
        [res.results[i]["outT"].reshape(NCH, D_OUT, PTS) for i in range(N_CORES)],
        axis=0,
    )  # (N_sel, D_out, PTS)
    return np.ascontiguousarray(outT.transpose(0, 2, 1))


# revision 24
# speedup vs baseline: 2.5723x; 1.4666x over previous
"""Trainium2 Bass kernel for AdaptiveLinearWithChannel (moe_routing).

Reference computation:
    w = weight[indices, t]          # (N_sel, D_in, D_out)
    b = bias[indices, t]            # (N_sel, 1, D_out)
    out = x @ w + b                 # (N_sel, PTS, D_out)

Sharding: the selected-channel dim N_sel=256 is split across 8 NeuronCores
(32 channels each, expert-parallel).  The per-channel weight/bias gather is
part of host-side sharding prep; each core then runs 32 independent
(2048x256)@(256x256) GEMMs + bias.

Device layout: the TensorEngine contracts along the partition axis, so x is
staged per-channel as x.T (D_in on partitions).  Each matmul computes an
out.T tile [D_out=128, pts=512] in PSUM (w-slice stationary, x.T moving),
bias is added by VectorE on the way out of PSUM, and the kernel writes out.T
per channel; the host transposes back when unsharding.
"""

import sys

import numpy as np

try:
    import concourse.bacc as bacc
except ImportError:  # fresh dir without the nix sitecustomize on sys.path
    sys.path.insert(0, "/opt/trn_rl_repo")
    import concourse.bacc as bacc

import concourse.mybir as mybir
import concourse.tile as tile
from concourse.bass_utils import run_bass_kernel_spmd

N_SEL = 256
PTS = 2048
D_IN = 256
D_OUT = 256
N_CORES = 8
NCH = N_SEL // N_CORES  # channels per core
P = 128  # partitions

# Compute mode: "f32" (exact), "f32r" (tf32-rate fp32), "bf16", "f16"
COMPUTE = "f16"
TRACE = False  # test.py flips this to get exec_time_ns

LAST_EXEC_TIME_NS = None

_CACHE = {}


def _mm_dtype(compute: str):
    return {
        "f32": mybir.dt.float32,
        "f32r": mybir.dt.float32r,
        "bf16": mybir.dt.bfloat16,
        "f16": mybir.dt.float16,
    }[compute]


def _build(compute: str):
    f32 = mybir.dt.float32
    mm_dt = _mm_dtype(compute)
    # fp16 mode also stores the output as fp16 (host upcasts exactly) --
    # halves the dominant DMA stream at ~2^-11 quantization error
    out_dt = mybir.dt.float16 if compute == "f16" else f32

    KH = D_IN // P  # 2 contraction halves
    MH = D_OUT // P  # 2 output-partition halves
    NPC = PTS // 512  # 4 moving chunks of 512
    nc = bacc.Bacc(None, target_bir_lowering=False)
    # x transposed, [kh, p, ch, pts]: channel-pair loads then read 2*PTS
    # contiguous bytes per partition
    xT_ext = nc.declare_dram_parameter("xT", [KH, P, NCH, PTS], mm_dt, isOutput=False)
    # w laid out [kh, p, ch, dout] so the whole-table preload gets 32KB
    # contiguous runs per partition
    w_ext = nc.declare_dram_parameter("w", [KH, P, NCH, D_OUT], mm_dt, isOutput=False)
    bT_ext = nc.declare_dram_parameter("bT", [D_OUT, NCH], f32, isOutput=False)
    out_ext = nc.declare_dram_parameter(
        "outT", [NCH, MH, P, NPC, 512], out_dt, isOutput=True
    )

    # Channel group sizes per x DMA: small groups at the head (first stores
    # issue sooner) and tail (final stores drain sooner), big in the middle.
    GROUPS = [2, 2] + [6] * 4 + [2, 2]
    assert sum(GROUPS) == NCH

    with tile.TileContext(nc) as tc:
        with (
            tc.tile_pool(name="xp", bufs=2) as xpool,
            tc.tile_pool(name="wp", bufs=1) as wpool,
            tc.tile_pool(name="bp", bufs=1) as bpool,
            tc.tile_pool(name="op", bufs=6) as opool,
            tc.tile_pool(name="pp", bufs=2, space="PSUM") as pspool,
        ):
            # Preloads ride the SWDGE (gpsimd) queue so they never
            # head-of-line-block the streaming x loads on the sync HWDGE ring.
            # w is preloaded in CPAIR-channel groups so the first matmuls only
            # wait on the first small group, not the whole table.
            b_sb = bpool.tile([P, MH, NCH], f32, tag="b", name="b_sb")
            for mh in range(MH):
                nc.gpsimd.dma_start(b_sb[:, mh, :], bT_ext[mh * P : (mh + 1) * P, :])
            w_all = wpool.tile([P, KH, NCH, D_OUT], mm_dt, tag="w", name="w_all")

            ch0 = 0
            for pr, gsz in enumerate(GROUPS):
                csl = slice(ch0, ch0 + gsz)
                for kh in range(KH):
                    nc.gpsimd.dma_start(
                        w_all[:, kh, csl, :], w_ext[kh, :, csl, :]
                    )
                # x loads: sync HWDGE ring; gsz*PTS contiguous per partition
                x_sb = xpool.tile(
                    [P, KH, gsz, PTS], mm_dt, tag="x", name=f"x{pr}",
                    padded_shape=[P, KH, max(GROUPS), PTS],
                )
                for kh in range(KH):
                    nc.sync.dma_start(
                        x_sb[:, kh, :, :],
                        xT_ext[kh, :, csl, :],
                    )
                for ci in range(gsz):
                    ch = ch0 + ci
                    o_sb = opool.tile(
                        [P, MH, NPC, 512], out_dt, tag="o", name=f"o{ch}"
                    )
                    for mh in range(MH):
                        # one 4-bank PSUM tile per (ch, mh); bufs=2 ping-pongs
                        # across the 8 banks
                        ps4 = pspool.tile(
                            [P, NPC, 512], f32, tag="ps", name=f"ps{ch}_{mh}"
                        )
                        for kh in range(KH):
                            lhsT = w_all[:, kh, ch, mh * P : (mh + 1) * P]
                            for pc in range(NPC):
                                nc.tensor.matmul(
                                    ps4[:, pc, :],
                                    lhsT,
                                    x_sb[:, kh, ci, pc * 512 : (pc + 1) * 512],
                                    start=(kh == 0),
                                    stop=(kh == KH - 1),
                                )
                        # bias add + PSUM evacuation, one 2048-wide op;
                        # alternate VectorE / ScalarE so neither binds
                        bcol = b_sb[:, mh, ch : ch + 1]
                        if mh == 0:
                            nc.vector.tensor_scalar_add(
                                o_sb[:, mh, :, :], ps4[:, :, :], bcol
                            )
                        else:
                            nc.scalar.add(o_sb[:, mh, :, :], ps4[:, :, :], bcol)
                    # one store per channel: scalar HWDGE ring (decoupled from
                    # loads); 1MB contiguous DRAM region, 4KB runs per partition
                    nc.scalar.dma_start(
                        out_ext[ch].transpose([1, 0, 2, 3]), o_sb[:, :, :, :]
                    )
                ch0 += gsz

    nc.compile()
    return nc


def _install_ntff_hook():
    """The agent image's antenv lacks axon_hooks; register the NTFF
    profiling hook ourselves so trace=True yields exec_time_ns."""
    try:
        from antenv.axon_hooks import get_axon_ntff_profile_hook  # noqa: F401

        return
    except ImportError:
        pass
    import types

    from trn_agent_boot.trn_boot import _ntff_profile_via_ctypes

    hook = _ntff_profile_via_ctypes("/opt/axon/libaxon_pjrt.so")
    mod = types.ModuleType("antenv.axon_hooks")
    mod.get_axon_ntff_profile_hook = lambda: hook
    mod.set_axon_ntff_profile_hook = lambda h: None
    sys.modules["antenv.axon_hooks"] = mod


def _round_tf32(a):
    """Round-to-nearest-even to the 10-bit mantissa the PE's FP32r
    (tfloat32) mode multiplies at."""
    u = a.view(np.uint32)
    r = (u + np.uint32(0xFFF) + ((u >> np.uint32(13)) & np.uint32(1))) & np.uint32(
        0xFFFFE000
    )
    return r.view(np.float32)


def kernel(x, weight, bias, indices, t):
    global LAST_EXEC_TIME_NS

    x = np.asarray(x, dtype=np.float32)
    weight = np.asarray(weight, dtype=np.float32)
    bias = np.asarray(bias, dtype=np.float32)
    idx = np.asarray(indices).astype(np.int64)
    t = int(np.asarray(t))

    # Host-side sharding prep: per-channel gather + transpose + dtype prep.
    w_sel = np.ascontiguousarray(weight[idx, t])  # (N_sel, D_in, D_out)
    b_sel = bias[idx, t, 0]  # (N_sel, D_out)
    if COMPUTE == "f32r":
        x = _round_tf32(x)
        w_sel = _round_tf32(w_sel)
    elif COMPUTE == "bf16":
        import ml_dtypes

        x = x.astype(ml_dtypes.bfloat16)
        w_sel = w_sel.astype(ml_dtypes.bfloat16)
    elif COMPUTE == "f16":
        x = x.astype(np.float16)
        w_sel = w_sel.astype(np.float16)

    in_maps = []
    for c in range(N_CORES):
        sl = slice(c * NCH, (c + 1) * NCH)
        # x device layout: [kh, p, ch, pts]
        x_dev = np.ascontiguousarray(x[sl].transpose(2, 0, 1)).reshape(
            D_IN // P, P, NCH, PTS
        )
        # w device layout: [kh, p, ch, dout]
        w_dev = np.ascontiguousarray(w_sel[sl].transpose(1, 0, 2)).reshape(
            D_IN // P, P, NCH, D_OUT
        )
        in_maps.append(
            {
                "xT": x_dev,
                "w": w_dev,
                "bT": np.ascontiguousarray(b_sel[sl].T),
            }
        )

    if COMPUTE not in _CACHE:
        _CACHE[COMPUTE] = _build(COMPUTE)
    nc = _CACHE[COMPUTE]

    if TRACE:
        _install_ntff_hook()
    res = run_bass_kernel_spmd(
        nc, in_maps, core_ids=list(range(N_CORES)), trace=TRACE
    )
    LAST_EXEC_TIME_NS = res.exec_time_ns

    outT = np.concatenate(
        [res.results[i]["outT"].reshape(NCH, D_OUT, PTS) for i in range(N_CORES)],
        axis=0,
    )  # (N_sel, D_out, PTS)
    out = np.ascontiguousarray(outT.transpose(0, 2, 1))
    if out.dtype != np.float32:
        out = out.astype(np.float32)
    return out
